# revision 3
# baseline (speedup 1.0000x reference)
"""Trainium2 kernel for: out = tanh(x @ scatter_nd(nonzero_ind, kernel_vector, (20000, 4096)) + bias).

Strategy (8 NeuronCores):
  - Host builds the dense (20000, 4096) weight matrix from the COO triples
    (sharding prep), zero-padded to 4 contraction shards of 5120 rows.
  - Shard: contraction K x4, batch x2  ->  core c = (batch_half h, k_quarter q)
    computes partial[h,q] = x[h*1024:(h+1)*1024, qK] @ W[qK, :]  (1024 x 4096).
  - On device: x shard is transposed via the PE array (fp32 has no DMA-transpose)
    into a SBUF-resident [K=5120 x B=1024] operand; W streams through once;
    fp32r matmuls accumulate across all 40 K-tiles in 8 PSUM banks
    (psum bank b = batch tile b, moving free dim = 512 output units).
  - Host sums the 4 K-partials per batch half, adds bias, applies tanh.
"""

import numpy as np

P = 128
B, K, U = 2048, 20000, 4096
KSPLIT, HSPLIT = 4, 2
KPAD = 5120              # per-K-shard rows, zero padded (4 * 5120 = 20480 >= 20000)
KT = KPAD // P           # 40 k-tiles per shard
B_SH = B // HSPLIT       # 1024 batch rows per core
NBT = B_SH // P          # 8 batch tiles -> 8 PSUM banks
UBLK = 512               # moving free dim per matmul
NUB = U // UBLK          # 8 unit blocks

TRACE = False            # set by test harness for profiled runs
LAST_RESULT = None       # BassKernelResults of the last run (for the harness)

_NC_CACHE = {}


def _build_nc(kpad=KPAD, b_sh=B_SH, u=U):
    from concourse import bacc
    import concourse.mybir as mybir
    import concourse.tile as tile
    from concourse.masks import make_identity

    f32 = mybir.dt.float32
    f32r = mybir.dt.float32r
    kt_n = kpad // P
    nbt = b_sh // P
    nub = u // UBLK

    nc = bacc.Bacc("TRN2", target_bir_lowering=False, debug=False)
    x_d = nc.dram_tensor("x_sh", [b_sh, kpad], f32, kind="ExternalInput").ap()
    w_d = nc.dram_tensor("w_sh", [kpad, u], f32, kind="ExternalInput").ap()
    o_d = nc.dram_tensor("out_p", [b_sh, u], f32, kind="ExternalOutput").ap()

    with tile.TileContext(nc) as tc:
        with tc.tile_pool(name="resid", bufs=1) as respool:
            ident = respool.tile([P, P], f32, tag="ident", name="ident")
            make_identity(nc, ident)
            xt = [
                respool.tile([P, b_sh], f32r, tag=f"xt{kt}", name=f"xt{kt}")
                for kt in range(kt_n)
            ]

            # Phase 1: x shard -> PE transpose -> resident SBUF [k, b] tiles.
            with (
                tc.tile_pool(name="xload", bufs=4) as xpool,
                tc.tile_pool(name="tpsum", bufs=4, space="PSUM") as tpsum,
            ):
                for kt in range(kt_n):
                    for bi in range(nbt):
                        xtile = xpool.tile([P, P], f32, tag="xtile", name="xtile")
                        nc.sync.dma_start(
                            xtile[:], x_d[bi * P:(bi + 1) * P, kt * P:(kt + 1) * P]
                        )
                        ps = tpsum.tile([P, P], f32, tag="tp", name="tp")
                        nc.tensor.transpose(ps[:], xtile[:], ident[:])
                        nc.vector.tensor_copy(xt[kt][:, bi * P:(bi + 1) * P], ps[:])

            # Phase 2: out[b, u] += xT[k, b].T @ W[k, u], accumulated over all
            # k-tiles in PSUM bank bi, streamed over 512-wide unit blocks.
            with (
                tc.tile_pool(name="wpool", bufs=8) as wpool,
                tc.tile_pool(name="stage", bufs=4) as spool,
                tc.tile_pool(name="mpsum", bufs=1, space="PSUM") as mpsum,
            ):
                for ub in range(nub):
                    psums = [
                        mpsum.tile([P, UBLK], f32, tag=f"ps{bi}", name=f"ps{bi}")
                        for bi in range(nbt)
                    ]
                    for kt in range(kt_n):
                        wt = wpool.tile([P, UBLK], f32r, tag="wt", name="wt")
                        nc.sync.dma_start(
                            wt[:],
                            w_d[kt * P:(kt + 1) * P, ub * UBLK:(ub + 1) * UBLK]
                            .bitcast(f32r),
                        )
                        for bi in range(nbt):
                            nc.tensor.matmul(
                                psums[bi][:],
                                xt[kt][:, bi * P:(bi + 1) * P],
                                wt[:],
                                start=(kt == 0),
                                stop=(kt == kt_n - 1),
                            )
                    for bi in range(nbt):
                        st = spool.tile([P, UBLK], f32, tag="st", name="st")
                        nc.vector.tensor_copy(st[:], psums[bi][:])
                        nc.sync.dma_start(
                            o_d[bi * P:(bi + 1) * P, ub * UBLK:(ub + 1) * UBLK],
                            st[:],
                        )

    nc.compile()
    return nc


def _get_nc(key=("full",), **kw):
    if key not in _NC_CACHE:
        _NC_CACHE[key] = _build_nc(**kw)
    return _NC_CACHE[key]


def kernel(x, kernel_vector, bias, nonzero_ind):
    global LAST_RESULT
    from concourse.bass_utils import run_bass_kernel_spmd

    x = np.asarray(x, dtype=np.float32)
    kernel_vector = np.asarray(kernel_vector, dtype=np.float32)
    bias = np.asarray(bias, dtype=np.float32)
    nonzero_ind = np.asarray(nonzero_ind)

    nc = _get_nc()

    # Host scatter: dense weights, rows padded to KSPLIT * KPAD.
    rows = nonzero_ind[:, 0].astype(np.int64)
    cols = nonzero_ind[:, 1].astype(np.int64)
    w_full = np.zeros(KSPLIT * KPAD * U, np.float32)
    np.add.at(w_full, rows * U + cols, kernel_vector)
    w_full = w_full.reshape(KSPLIT * KPAD, U)

    in_maps = []
    for c in range(8):
        h, q = divmod(c, KSPLIT)
        k0 = q * KPAD
        k1 = min(K, k0 + KPAD)
        xs = np.zeros((B_SH, KPAD), np.float32)
        xs[:, : k1 - k0] = x[h * B_SH:(h + 1) * B_SH, k0:k1]
        in_maps.append({"x_sh": xs, "w_sh": w_full[k0:k0 + KPAD]})

    kwargs = {}
    if TRACE:
        kwargs = dict(trace=True, trace_cores=list(range(8)))
    res = run_bass_kernel_spmd(nc, in_maps, core_ids=list(range(8)), **kwargs)
    LAST_RESULT = res

    out = np.empty((B, U), np.float32)
    for h in range(HSPLIT):
        acc = res.results[h * KSPLIT]["out_p"].copy()
        for q in range(1, KSPLIT):
            acc += res.results[h * KSPLIT + q]["out_p"]
        out[h * B_SH:(h + 1) * B_SH] = np.tanh(acc + bias[None, :])
    return out


# revision 4
# speedup vs baseline: 1.3780x; 1.3780x over previous
"""Trainium2 kernel for: out = tanh(x @ scatter_nd(nonzero_ind, kernel_vector, (20000, 4096)) + bias).

Strategy (8 NeuronCores):
  - Host builds the dense (20000, 4096) weight matrix from the COO triples and
    pre-transposes x shards (sharding prep), zero-padded to 4 contraction
    shards of 5120 rows.
  - Shard: contraction K x4, batch x2  ->  core c = (batch_half h, k_quarter q)
    computes partial[h,q] = x[h*1024:(h+1)*1024, qK] @ W[qK, :]  (1024 x 4096).
  - On device: the transposed x shard (5120 x 1024) lives SBUF-resident as 40
    [128 x 1024] fp32r tiles (stationary matmul operand); W streams through
    once; fp32r matmuls (full PE rate at moving dim 512) accumulate across all
    40 K-tiles in 8 PSUM banks (bank b = batch tile b).
  - Host sums the 4 K-partials per batch half, adds bias, applies tanh.
"""

import numpy as np

P = 128
B, K, U = 2048, 20000, 4096
KSPLIT, HSPLIT = 4, 2
KPAD = 5120              # per-K-shard rows, zero padded (4 * 5120 = 20480 >= 20000)
KT = KPAD // P           # 40 k-tiles per shard
B_SH = B // HSPLIT       # 1024 batch rows per core
NBT = B_SH // P          # 8 batch tiles -> 8 PSUM banks
UBLK = 512               # moving free dim per matmul
NUB = U // UBLK          # 8 unit blocks

TRACE = False            # set by test harness for profiled runs
LAST_RESULT = None       # BassKernelResults of the last run (for the harness)

_NC_CACHE = {}


def _build_nc(kpad=KPAD, b_sh=B_SH, u=U):
    from concourse import bacc
    import concourse.mybir as mybir
    import concourse.tile as tile

    f32 = mybir.dt.float32
    f32r = mybir.dt.float32r
    kt_n = kpad // P
    nbt = b_sh // P
    nub = u // UBLK

    nc = bacc.Bacc("TRN2", target_bir_lowering=False, debug=False)
    xt_d = nc.dram_tensor("xt_sh", [kpad, b_sh], f32, kind="ExternalInput").ap()
    w_d = nc.dram_tensor("w_sh", [kpad, u], f32, kind="ExternalInput").ap()
    o_d = nc.dram_tensor("out_p", [b_sh, u], f32, kind="ExternalOutput").ap()

    with tile.TileContext(nc) as tc:
        with (
            tc.tile_pool(name="resid", bufs=1) as respool,
            tc.tile_pool(name="wpool", bufs=8) as wpool,
            tc.tile_pool(name="stage", bufs=8) as spool,
            tc.tile_pool(name="mpsum", bufs=1, space="PSUM") as mpsum,
        ):
            # Resident transposed-x tiles, loaded straight from DRAM.
            xt = []
            for kt in range(kt_n):
                xtile = respool.tile([P, b_sh], f32r, tag=f"xt{kt}", name=f"xt{kt}")
                nc.sync.dma_start(
                    xtile[:], xt_d[kt * P:(kt + 1) * P, :].bitcast(f32r)
                )
                xt.append(xtile)

            # out[b, u] += xT[k, b].T @ W[k, u], accumulated over all k-tiles
            # in PSUM bank bi, streamed over 512-wide unit blocks.
            for ub in range(nub):
                psums = [
                    mpsum.tile([P, UBLK], f32, tag=f"ps{bi}", name=f"ps{bi}")
                    for bi in range(nbt)
                ]
                for kt in range(kt_n):
                    wt = wpool.tile([P, UBLK], f32r, tag="wt", name="wt")
                    nc.sync.dma_start(
                        wt[:],
                        w_d[kt * P:(kt + 1) * P, ub * UBLK:(ub + 1) * UBLK]
                        .bitcast(f32r),
                    )
                    for bi in range(nbt):
                        nc.tensor.matmul(
                            psums[bi][:],
                            xt[kt][:, bi * P:(bi + 1) * P],
                            wt[:],
                            start=(kt == 0),
                            stop=(kt == kt_n - 1),
                        )
                for bi in range(nbt):
                    st = spool.tile([P, UBLK], f32, tag="st", name="st")
                    nc.vector.tensor_copy(st[:], psums[bi][:])
                    nc.sync.dma_start(
                        o_d[bi * P:(bi + 1) * P, ub * UBLK:(ub + 1) * UBLK],
                        st[:],
                    )

    nc.compile()
    return nc


def _get_nc(key=("full",), **kw):
    if key not in _NC_CACHE:
        _NC_CACHE[key] = _build_nc(**kw)
    return _NC_CACHE[key]


def kernel(x, kernel_vector, bias, nonzero_ind):
    global LAST_RESULT
    from concourse.bass_utils import run_bass_kernel_spmd

    x = np.asarray(x, dtype=np.float32)
    kernel_vector = np.asarray(kernel_vector, dtype=np.float32)
    bias = np.asarray(bias, dtype=np.float32)
    nonzero_ind = np.asarray(nonzero_ind)

    nc = _get_nc()

    # Host scatter: dense weights, rows padded to KSPLIT * KPAD.
    rows = nonzero_ind[:, 0].astype(np.int64)
    cols = nonzero_ind[:, 1].astype(np.int64)
    w_full = np.zeros(KSPLIT * KPAD * U, np.float32)
    np.add.at(w_full, rows * U + cols, kernel_vector)
    w_full = w_full.reshape(KSPLIT * KPAD, U)

    in_maps = []
    for c in range(8):
        h, q = divmod(c, KSPLIT)
        k0 = q * KPAD
        k1 = min(K, k0 + KPAD)
        xs = np.zeros((KPAD, B_SH), np.float32)
        xs[: k1 - k0] = x[h * B_SH:(h + 1) * B_SH, k0:k1].T
        in_maps.append({"xt_sh": xs, "w_sh": w_full[k0:k0 + KPAD]})

    kwargs = {}
    if TRACE:
        kwargs = dict(trace=True, trace_cores=list(range(8)))
    res = run_bass_kernel_spmd(nc, in_maps, core_ids=list(range(8)), **kwargs)
    LAST_RESULT = res

    out = np.empty((B, U), np.float32)
    for h in range(HSPLIT):
        acc = res.results[h * KSPLIT]["out_p"].copy()
        for q in range(1, KSPLIT):
            acc += res.results[h * KSPLIT + q]["out_p"]
        out[h * B_SH:(h + 1) * B_SH] = np.tanh(acc + bias[None, :])
    return out


# revision 5
# speedup vs baseline: 1.4387x; 1.0441x over previous
"""Trainium2 kernel for: out = tanh(x @ scatter_nd(nonzero_ind, kernel_vector, (20000, 4096)) + bias).

Strategy (8 NeuronCores):
  - Host builds the dense (20000, 4096) weight matrix from the COO triples and
    pre-transposes x shards (sharding prep), zero-padded to 4 contraction
    shards of 5120 rows.
  - Shard: contraction K x4, batch x2  ->  core c = (batch_half h, k_quarter q)
    computes partial[h,q] = x[h*1024:(h+1)*1024, qK] @ W[qK, :]  (1024 x 4096).
  - On device: the transposed x shard (5120 x 1024) lives SBUF-resident as 40
    [128 x 1024] fp32r tiles (stationary matmul operand); W streams through
    once; fp32r matmuls (full PE rate at moving dim 512) accumulate across all
    40 K-tiles in 8 PSUM banks (bank b = batch tile b).
  - Host sums the 4 K-partials per batch half, adds bias, applies tanh.
"""

import numpy as np

P = 128
B, K, U = 2048, 20000, 4096
KSPLIT, HSPLIT = 4, 2
KPAD = 5120              # per-K-shard rows, zero padded (4 * 5120 = 20480 >= 20000)
KT = KPAD // P           # 40 k-tiles per shard
B_SH = B // HSPLIT       # 1024 batch rows per core
NBT = B_SH // P          # 8 batch tiles -> 8 PSUM banks
UBLK = 512               # moving free dim per matmul
NUB = U // UBLK          # 8 unit blocks

TRACE = False            # set by test harness for profiled runs
LAST_RESULT = None       # BassKernelResults of the last run (for the harness)

_NC_CACHE = {}


def _build_nc(kpad=KPAD, b_sh=B_SH, u=U):
    from concourse import bacc
    import concourse.mybir as mybir
    import concourse.tile as tile

    f32 = mybir.dt.float32
    f32r = mybir.dt.float32r
    kt_n = kpad // P
    nbt = b_sh // P
    nub = u // UBLK

    nc = bacc.Bacc("TRN2", target_bir_lowering=False, debug=False)
    xt_d = nc.dram_tensor("xt_sh", [kpad, b_sh], f32, kind="ExternalInput").ap()
    w_d = nc.dram_tensor("w_sh", [kpad, u], f32, kind="ExternalInput").ap()
    o_d = nc.dram_tensor("out_p", [b_sh, u], f32, kind="ExternalOutput").ap()

    with tile.TileContext(nc) as tc:
        with (
            tc.tile_pool(name="resid", bufs=1) as respool,
            tc.tile_pool(name="wpool", bufs=8) as wpool,
            tc.tile_pool(name="stage", bufs=8) as spool,
            tc.tile_pool(name="mpsum", bufs=1, space="PSUM") as mpsum,
        ):
            # Resident transposed-x tiles; DMAs are emitted interleaved with
            # the first unit block's k-loop so the PE pipeline fills
            # immediately instead of waiting behind the whole 21MB x load.
            xt = [
                respool.tile([P, b_sh], f32r, tag=f"xt{kt}", name=f"xt{kt}")
                for kt in range(kt_n)
            ]

            # out[b, u] += xT[k, b].T @ W[k, u], accumulated over all k-tiles
            # in PSUM bank bi, streamed over 512-wide unit blocks.
            for ub in range(nub):
                psums = [
                    mpsum.tile([P, UBLK], f32, tag=f"ps{bi}", name=f"ps{bi}")
                    for bi in range(nbt)
                ]
                for kt in range(kt_n):
                    if ub == 0:
                        nc.sync.dma_start(
                            xt[kt][:], xt_d[kt * P:(kt + 1) * P, :].bitcast(f32r)
                        )
                    wt = wpool.tile([P, UBLK], f32r, tag="wt", name="wt")
                    nc.sync.dma_start(
                        wt[:],
                        w_d[kt * P:(kt + 1) * P, ub * UBLK:(ub + 1) * UBLK]
                        .bitcast(f32r),
                    )
                    for bi in range(nbt):
                        nc.tensor.matmul(
                            psums[bi][:],
                            xt[kt][:, bi * P:(bi + 1) * P],
                            wt[:],
                            start=(kt == 0),
                            stop=(kt == kt_n - 1),
                        )
                for bi in range(nbt):
                    st = spool.tile([P, UBLK], f32, tag="st", name="st")
                    nc.vector.tensor_copy(st[:], psums[bi][:])
                    nc.sync.dma_start(
                        o_d[bi * P:(bi + 1) * P, ub * UBLK:(ub + 1) * UBLK],
                        st[:],
                    )

    nc.compile()
    return nc


def _get_nc(key=("full",), **kw):
    if key not in _NC_CACHE:
        _NC_CACHE[key] = _build_nc(**kw)
    return _NC_CACHE[key]


def kernel(x, kernel_vector, bias, nonzero_ind):
    global LAST_RESULT
    from concourse.bass_utils import run_bass_kernel_spmd

    x = np.asarray(x, dtype=np.float32)
    kernel_vector = np.asarray(kernel_vector, dtype=np.float32)
    bias = np.asarray(bias, dtype=np.float32)
    nonzero_ind = np.asarray(nonzero_ind)

    nc = _get_nc()

    # Host scatter: dense weights, rows padded to KSPLIT * KPAD.
    rows = nonzero_ind[:, 0].astype(np.int64)
    cols = nonzero_ind[:, 1].astype(np.int64)
    w_full = np.zeros(KSPLIT * KPAD * U, np.float32)
    np.add.at(w_full, rows * U + cols, kernel_vector)
    w_full = w_full.reshape(KSPLIT * KPAD, U)

    in_maps = []
    for c in range(8):
        h, q = divmod(c, KSPLIT)
        k0 = q * KPAD
        k1 = min(K, k0 + KPAD)
        xs = np.zeros((KPAD, B_SH), np.float32)
        xs[: k1 - k0] = x[h * B_SH:(h + 1) * B_SH, k0:k1].T
        in_maps.append({"xt_sh": xs, "w_sh": w_full[k0:k0 + KPAD]})

    kwargs = {}
    if TRACE:
        kwargs = dict(trace=True, trace_cores=list(range(8)))
    res = run_bass_kernel_spmd(nc, in_maps, core_ids=list(range(8)), **kwargs)
    LAST_RESULT = res

    out = np.empty((B, U), np.float32)
    for h in range(HSPLIT):
        acc = res.results[h * KSPLIT]["out_p"].copy()
        for q in range(1, KSPLIT):
            acc += res.results[h * KSPLIT + q]["out_p"]
        out[h * B_SH:(h + 1) * B_SH] = np.tanh(acc + bias[None, :])
    return out
